# revision 9
# baseline (speedup 1.0000x reference)
"""NHGAT-VAE (GAT heads + GAT + 2 GCN layers + inner-product decoder) on 8 trn2 cores.

Sharding: core k owns rows [k*512, (k+1)*512). Host passes:
  - adjT  = adj[rows_k].T           [4096, 512]  (mask AND spmm-lhsT layout)
  - xT    = x.T (replicated)        [512, 4096]
  - xTl   = x[rows_k].T             [512, 512]
  - rhsh  = [W_h | W_h@a2_h]        [4, 512, 65]
  - wa1   = [W_h@a1_h cols]         [512, 4]
  - rhsa  = [W_att | W@a2 | W@a1]   [256, 66]
  - w1, w23=[W2|W3], epsl

Device per core: replicated Wh/s1/s2 compute; fused GAT passes where
P = exp(leakyrelu(s1[r]+s2[c])) * adjT accumulates [num|_|den] in PSUM over 32
c-chunks (softmax w/o max-subtract: scores are O(1)); 4 AllGathers chain the
stages; z@z.T row-block decoded locally.
"""
import numpy as np

N, D, F1, F2, F3, H = 4096, 512, 64, 32, 16, 4
NCORES = 8
R = N // NCORES      # 512 rows per core
NCH = N // 128       # 32 c-chunks
RT = R // 128        # 4 local row tiles
ALPHA = 0.2

_CACHE = {}


def _emit(nc, tc, tile, mybir, masks, io):
    dt = mybir.dt.float32
    AF = mybir.ActivationFunctionType

    adjT, xT, xTl, rhsh, wa1, rhsa, w1, w23, epsl = (
        io["adjT"], io["xT"], io["xTl"], io["rhsh"], io["wa1"], io["rhsa"],
        io["w1"], io["w23"], io["epsl"])
    out_rec, out_mu, out_lv = io["out_rec"], io["out_mu"], io["out_lv"]
    out_cat, out_gc = io["out_cat"], io["out_gc"]

    import contextlib
    ctx = contextlib.ExitStack()
    with ctx:
        pa = ctx.enter_context(tc.tile_pool(name="persist", bufs=1))
        pw = ctx.enter_context(tc.tile_pool(name="work", bufs=3))
        pacc = ctx.enter_context(tc.tile_pool(name="acc", bufs=1, space="PSUM"))
        ptr = ctx.enter_context(tc.tile_pool(name="ptrans", bufs=3, space="PSUM"))
        dram = ctx.enter_context(tc.tile_pool(name="dram", bufs=1, space="DRAM"))

        # ---- persistent small constants ----
        ident = pa.tile([128, 128], dt, tag="ident")
        masks.make_identity(nc, ident[:])
        ones1 = pa.tile([1, 128], dt, tag="ones1")
        nc.vector.memset(ones1[:], 1.0)

        # ---- persistent big arrays ----
        mask = [pa.tile([128, R], dt, tag=f"mask{i}", name=f"mask{i}") for i in range(NCH)]
        for i in range(NCH):
            nc.sync.dma_start(mask[i][:], adjT[128 * i:128 * (i + 1), :])
        whs = [pa.tile([128, 66 * NCH], dt, tag=f"whs{h}", name=f"whs{h}") for h in range(H)]
        s1b = [pa.tile([128, R], dt, tag=f"s1b{h}", name=f"s1b{h}") for h in range(H)]
        s1row = [pa.tile([1, R], dt, tag=f"s1row{h}", name=f"s1row{h}")
                 for h in range(H)]
        c2 = [pa.tile([64, R], dt, tag=f"c2_{hq}", name=f"c2_{hq}") for hq in range(H)]

        _bign = [0]

        def bigp(shape=(128, 512)):
            _bign[0] += 1
            return ptr.tile(list(shape), dt, tag="big", name=f"big{_bign[0]}")

        def t_copy(dst, src):
            nc.vector.tensor_copy(dst, src)

        # ================= stage 0: Wh/s1/s2 for all N (replicated) ========
        with tc.tile_pool(name="st0", bufs=1) as p0:
            rhb = [[p0.tile([128, 65], dt, tag=f"rhb{h}_{j}", name=f"rhb{h}_{j}") for j in range(4)]
                   for h in range(H)]
            for h in range(H):
                for j in range(4):
                    nc.sync.dma_start(rhb[h][j][:], rhsh[h, 128 * j:128 * (j + 1), :])
            wab = [p0.tile([128, H], dt, tag=f"wab{j}", name=f"wab{j}") for j in range(4)]
            for j in range(4):
                nc.sync.dma_start(wab[j][:], wa1[128 * j:128 * (j + 1), :])
            xtl = [p0.tile([128, R], dt, tag=f"xtl{j}", name=f"xtl{j}") for j in range(4)]
            for j in range(4):
                nc.sync.dma_start(xtl[j][:], xTl[128 * j:128 * (j + 1), :])

            # s1 for local rows: [512, H] -> per-head s1row [1, 512]
            for rt in range(RT):
                ps4 = bigp((128, H))
                for j in range(4):
                    nc.tensor.matmul(ps4[:], xtl[j][:, 128 * rt:128 * (rt + 1)],
                                     wab[j][:], start=(j == 0), stop=(j == 3))
                s4 = pw.tile([128, H], dt, tag="s4")
                t_copy(s4[:], ps4[:])
                for h in range(H):
                    psT = bigp((1, 128))
                    nc.tensor.transpose(psT[:], s4[:, h:h + 1], ident[:])
                    t_copy(s1row[h][0:1, 128 * rt:128 * (rt + 1)], psT[:])

            # broadcast s1 rows -> [128, 512] per head
            for h in range(H):
                pb = bigp()
                nc.tensor.matmul(pb[:], ones1[:], s1row[h][:],
                                 start=True, stop=True)
                t_copy(s1b[h][:], pb[:])

            # Wh/s2 for all chunks, two column halves of xT
            for hf in range(2):
                xt = [p0.tile([128, 2048], dt, tag=f"xt{j}", name=f"xt{j}_{hf}") for j in range(4)]
                for j in range(4):
                    nc.sync.dma_start(
                        xt[j][:], xT[128 * j:128 * (j + 1),
                                     2048 * hf:2048 * (hf + 1)])
                for ci in range(16):
                    i = hf * 16 + ci
                    for h in range(H):
                        p66 = bigp((128, 65))
                        for j in range(4):
                            nc.tensor.matmul(
                                p66[:], xt[j][:, 128 * ci:128 * (ci + 1)],
                                rhb[h][j][:], start=(j == 0), stop=(j == 3))
                        t_copy(whs[h][:, 66 * i:66 * i + 65], p66[:])
            for h in range(H):
                nc.vector.memset(whs[h][:, 65::66], 1.0)

        # ================= stage 1: 4 GAT heads, fused pass =================
        acc = [pacc.tile([128, 66 * H], dt, tag=f"acc{rt}", name=f"acc1_{rt}") for rt in range(RT)]
        zrow = pa.tile([1, 66 * H], dt, tag="zrow")
        nc.vector.memset(zrow[:], 0.0)
        for rt in range(RT):
            nc.tensor.matmul(acc[rt][:], ones1[:, 0:128], zrow[:],
                             start=True, stop=False, skip_group_check=True)
        for i in range(NCH):
            for h in range(H):
                tl = pw.tile([128, R], dt, tag="tlr")
                nc.scalar.activation(tl[:], s1b[h][:], AF.Prelu,
                                     bias=whs[h][:, 66 * i + 64:66 * i + 65],
                                     scale=1.0, alpha=ALPHA)
                tp = pw.tile([128, R], dt, tag="texp")
                nc.scalar.activation(tp[:], tl[:], AF.Exp)
                pm = pw.tile([128, R], dt, tag="tmask")
                nc.vector.tensor_mul(pm[:], tp[:], mask[i][:])
                for rt in range(RT):
                    nc.tensor.matmul(
                        acc[rt][:, 66 * h:66 * (h + 1)],
                        pm[:, 128 * rt:128 * (rt + 1)],
                        whs[h][:, 66 * i:66 * (i + 1)],
                        start=False, stop=(i == NCH - 1),
                        skip_group_check=True)

        for rt in range(RT):
            accs = pw.tile([128, 66 * H], dt, tag="accs", name=f"accs{rt}")
            t_copy(accs[:], acc[rt][:])
            nc.sync.dma_start(io["out_acc"][128 * rt:128 * (rt + 1), :], accs[:])

        # epilogue: h = elu(num/den); build catT in c2; dump cat rows (debug)
        def gat_epilogue(acc_ap, h, rt, dst_c2, dst_dbg):
            num = acc_ap[:, 66 * h:66 * h + 64]
            den = acc_ap[:, 66 * h + 65:66 * h + 66]
            rec = pw.tile([128, 1], dt, tag="rec")
            nc.vector.reciprocal(rec[:], den)
            v = pw.tile([128, 64], dt, tag="vnorm")
            nc.vector.tensor_scalar_mul(v[:], num, rec[:])
            e1 = pw.tile([128, 64], dt, tag="e1")
            nc.scalar.activation(e1[:], v[:], AF.Exp)
            e2 = pw.tile([128, 64], dt, tag="e2")
            nc.scalar.activation(e2[:], e1[:], AF.Relu, bias=1.0, scale=-1.0)
            e3 = pw.tile([128, 64], dt, tag="e3")
            nc.scalar.activation(e3[:], v[:], AF.Relu)
            hh = pw.tile([128, 64], dt, tag="hh")
            nc.vector.tensor_sub(hh[:], e3[:], e2[:])
            pt = bigp((64, 128))
            nc.tensor.transpose(pt[:], hh[:], ident[:])
            t_copy(dst_c2, pt[:])
            if dst_dbg is not None:
                nc.sync.dma_start(dst_dbg, hh[:])

        for h in range(H):
            for rt in range(RT):
                gat_epilogue(acc[rt][:], h, rt,
                             c2[h][:, 128 * rt:128 * (rt + 1)],
                             out_cat[128 * rt:128 * (rt + 1), 64 * h:64 * (h + 1)])

        # ============ stage 1.5: Wh2/s2_2/s1_2 local + AllGather ============
        with tc.tile_pool(name="late", bufs=1) as pl:
            rab = [pl.tile([64, 66], dt, tag=f"rab{j}", name=f"rab{j}") for j in range(H)]
            for j in range(H):
                nc.sync.dma_start(rab[j][:], rhsa[64 * j:64 * (j + 1), :])
            agi2 = dram.tile([R, 65], dt, tag="agi2")
            ago2 = dram.tile([N, 65], dt, tag="ago2", addr_space="Shared")
            s1row2 = pl.tile([1, R], dt, tag="s1row2")
            for rt in range(RT):
                q = bigp((128, 66))
                for j in range(H):
                    nc.tensor.matmul(q[:], c2[j][:, 128 * rt:128 * (rt + 1)],
                                     rab[j][:], start=(j == 0), stop=(j == H - 1))
                qs = pw.tile([128, 66], dt, tag="qs")
                t_copy(qs[:], q[:])
                nc.sync.dma_start(agi2[128 * rt:128 * (rt + 1), :], qs[:, 0:65])
                pq = bigp((1, 128))
                nc.tensor.transpose(pq[:], qs[:, 65:66], ident[:])
                t_copy(s1row2[:, 128 * rt:128 * (rt + 1)], pq[:])
            nc.gpsimd.collective_compute(
                "AllGather", mybir.AluOpType.bypass,
                replica_groups=[list(range(NCORES))],
                ins=[agi2.opt()], outs=[ago2.opt()])

            s1b2 = pl.tile([128, R], dt, tag="s1b2")
            pb = bigp()
            nc.tensor.matmul(pb[:], ones1[:], s1row2[:], start=True, stop=True)
            t_copy(s1b2[:], pb[:])

            whs2 = pl.tile([128, 66 * NCH], dt, tag="whs2")
            for i in range(NCH):
                nc.sync.dma_start(whs2[:, 66 * i:66 * i + 65],
                                  ago2[128 * i:128 * (i + 1), :])
            nc.vector.memset(whs2[:, 65::66], 1.0)

            # ================= stage 2: attention GAT =================
            acc2 = [pacc.tile([128, 66], dt, tag=f"acc{rt}", name=f"acc2_{rt}") for rt in range(RT)]
            for i in range(NCH):
                tl = pw.tile([128, R], dt, tag="tlr")
                nc.scalar.activation(tl[:], s1b2[:], AF.Prelu,
                                     bias=whs2[:, 66 * i + 64:66 * i + 65],
                                     scale=1.0, alpha=ALPHA)
                tp = pw.tile([128, R], dt, tag="texp")
                nc.scalar.activation(tp[:], tl[:], AF.Exp)
                pm = pw.tile([128, R], dt, tag="tmask")
                nc.vector.tensor_mul(pm[:], tp[:], mask[i][:])
                for rt in range(RT):
                    nc.tensor.matmul(acc2[rt][:], pm[:, 128 * rt:128 * (rt + 1)],
                                     whs2[:, 66 * i:66 * (i + 1)],
                                     start=(i == 0), stop=(i == NCH - 1))

            gcT = pl.tile([64, R], dt, tag="gcT")
            for rt in range(RT):
                gat_epilogue(acc2[rt][:], 0, rt,
                             gcT[:, 128 * rt:128 * (rt + 1)],
                             out_gc[128 * rt:128 * (rt + 1), :])

            # ---- t1 = gc @ W1, AllGather ----
            w1b = pl.tile([64, F2], dt, tag="w1b")
            nc.sync.dma_start(w1b[:], w1[:, :])
            agi3a = dram.tile([R, F2], dt, tag="agi3a")
            ago3a = dram.tile([N, F2], dt, tag="ago3a", addr_space="Shared")
            for rt in range(RT):
                p1 = bigp((128, F2))
                nc.tensor.matmul(p1[:], gcT[:, 128 * rt:128 * (rt + 1)], w1b[:],
                                 start=True, stop=True)
                t1s = pw.tile([128, F2], dt, tag="t1s")
                t_copy(t1s[:], p1[:])
                nc.sync.dma_start(agi3a[128 * rt:128 * (rt + 1), :], t1s[:])
            nc.gpsimd.collective_compute(
                "AllGather", mybir.AluOpType.bypass,
                replica_groups=[list(range(NCORES))],
                ins=[agi3a.opt()], outs=[ago3a.opt()])

            # ================= stage 3: h1 = relu(adj @ t1) =================
            t1f = [pl.tile([128, F2], dt, tag=f"t1f{i}", name=f"t1f{i}") for i in range(NCH)]
            for i in range(NCH):
                nc.sync.dma_start(t1f[i][:], ago3a[128 * i:128 * (i + 1), :])
            acc3 = [pacc.tile([128, F2], dt, tag=f"acc{rt}", name=f"acc3_{rt}") for rt in range(RT)]
            for i in range(NCH):
                for rt in range(RT):
                    nc.tensor.matmul(acc3[rt][:], mask[i][:, 128 * rt:128 * (rt + 1)],
                                     t1f[i][:], start=(i == 0), stop=(i == NCH - 1))
            h1T = pl.tile([F2, R], dt, tag="h1T")
            w23b = pl.tile([F2, 2 * F3], dt, tag="w23b")
            nc.sync.dma_start(w23b[:], w23[:, :])
            agi3b = dram.tile([R, F2], dt, tag="agi3b")
            ago3b = dram.tile([N, F2], dt, tag="ago3b", addr_space="Shared")
            for rt in range(RT):
                h1s = pw.tile([128, F2], dt, tag="h1s")
                nc.scalar.activation(h1s[:], acc3[rt][:], AF.Relu)
                ph = bigp((F2, 128))
                nc.tensor.transpose(ph[:], h1s[:], ident[:])
                t_copy(h1T[:, 128 * rt:128 * (rt + 1)], ph[:])
            for rt in range(RT):
                p23 = bigp((128, 2 * F3))
                nc.tensor.matmul(p23[:], h1T[:, 128 * rt:128 * (rt + 1)], w23b[:],
                                 start=True, stop=True)
                t23s = pw.tile([128, 2 * F3], dt, tag="t23s")
                t_copy(t23s[:], p23[:])
                nc.sync.dma_start(agi3b[128 * rt:128 * (rt + 1), :], t23s[:])
            nc.gpsimd.collective_compute(
                "AllGather", mybir.AluOpType.bypass,
                replica_groups=[list(range(NCORES))],
                ins=[agi3b.opt()], outs=[ago3b.opt()])

            # ========== stage 4: mu/logvar local rows; z; AllGather z =======
            t23f = [pl.tile([128, 2 * F3], dt, tag=f"t23f{i}", name=f"t23f{i}") for i in range(NCH)]
            for i in range(NCH):
                nc.sync.dma_start(t23f[i][:], ago3b[128 * i:128 * (i + 1), :])
            acc4 = [pacc.tile([128, 2 * F3], dt, tag=f"acc{rt}", name=f"acc4_{rt}") for rt in range(RT)]
            for i in range(NCH):
                for rt in range(RT):
                    nc.tensor.matmul(acc4[rt][:], mask[i][:, 128 * rt:128 * (rt + 1)],
                                     t23f[i][:], start=(i == 0), stop=(i == NCH - 1))
            epst = [pl.tile([128, F3], dt, tag=f"epst{rt}", name=f"epst{rt}") for rt in range(RT)]
            for rt in range(RT):
                nc.sync.dma_start(epst[rt][:], epsl[128 * rt:128 * (rt + 1), :])
            agi4 = dram.tile([R, F3], dt, tag="agi4")
            ago4 = dram.tile([N, F3], dt, tag="ago4", addr_space="Shared")
            zTl = pl.tile([F3, R], dt, tag="zTl")
            for rt in range(RT):
                mus = pw.tile([128, F3], dt, tag="mus")
                t_copy(mus[:], acc4[rt][:, 0:F3])
                nc.sync.dma_start(out_mu[128 * rt:128 * (rt + 1), :], mus[:])
                lvs = pw.tile([128, F3], dt, tag="lvs")
                t_copy(lvs[:], acc4[rt][:, F3:2 * F3])
                nc.sync.dma_start(out_lv[128 * rt:128 * (rt + 1), :], lvs[:])
                elv = pw.tile([128, F3], dt, tag="elv")
                nc.scalar.activation(elv[:], acc4[rt][:, F3:2 * F3], AF.Exp)
                zm = pw.tile([128, F3], dt, tag="zm")
                nc.vector.tensor_mul(zm[:], elv[:], epst[rt][:])
                zz = pw.tile([128, F3], dt, tag="zz")
                nc.vector.tensor_add(zz[:], zm[:], acc4[rt][:, 0:F3])
                nc.sync.dma_start(agi4[128 * rt:128 * (rt + 1), :], zz[:])
                pzt = bigp((F3, 128))
                nc.tensor.transpose(pzt[:], zz[:], ident[:])
                t_copy(zTl[:, 128 * rt:128 * (rt + 1)], pzt[:])
            nc.gpsimd.collective_compute(
                "AllGather", mybir.AluOpType.bypass,
                replica_groups=[list(range(NCORES))],
                ins=[agi4.opt()], outs=[ago4.opt()])

            # ================= decode: adj_rec rows = z_loc @ z.T ===========
            zT = pl.tile([F3, N], dt, tag="zT")
            for i in range(NCH):
                zf = pw.tile([128, F3], dt, tag="zf")
                nc.sync.dma_start(zf[:], ago4[128 * i:128 * (i + 1), :])
                pzt = bigp((F3, 128))
                nc.tensor.transpose(pzt[:], zf[:], ident[:])
                t_copy(zT[:, 128 * i:128 * (i + 1)], pzt[:])
            for rt in range(RT):
                for cb in range(8):
                    pz = bigp()
                    nc.tensor.matmul(pz[:], zTl[:, 128 * rt:128 * (rt + 1)],
                                     zT[:, 512 * cb:512 * (cb + 1)],
                                     start=True, stop=True)
                    zo = pw.tile([128, 512], dt, tag="zo")
                    if (rt * 8 + cb) % 2 == 0:
                        nc.vector.tensor_copy(zo[:], pz[:])
                    else:
                        nc.scalar.activation(zo[:], pz[:], AF.Copy)
                    nc.sync.dma_start(
                        out_rec[128 * rt:128 * (rt + 1), 512 * cb:512 * (cb + 1)],
                        zo[:])


def _build():
    import concourse.bacc as bacc
    import concourse.tile as tile
    import concourse.mybir as mybir
    from concourse import masks
    dt = mybir.dt.float32

    nc = bacc.Bacc("TRN2", target_bir_lowering=False, debug=False,
                   num_devices=NCORES)
    io = {}
    for name, shape in [("adjT", [N, R]), ("xT", [D, N]), ("xTl", [D, R]),
                        ("rhsh", [H, D, 65]), ("wa1", [D, H]),
                        ("rhsa", [H * F1, 66]), ("w1", [F1, F2]),
                        ("w23", [F2, 2 * F3]), ("epsl", [R, F3])]:
        io[name] = nc.dram_tensor(name, shape, dt, kind="ExternalInput").ap()
    for name, shape in [("out_rec", [R, N]), ("out_mu", [R, F3]),
                        ("out_lv", [R, F3]), ("out_cat", [R, H * F1]),
                        ("out_gc", [R, F1]), ("out_acc", [R, 66 * H])]:
        io[name] = nc.dram_tensor(name, shape, dt, kind="ExternalOutput").ap()

    with tile.TileContext(nc) as tc:
        _emit(nc, tc, tile, mybir, masks, io)
    nc.compile()
    return nc


def _get_nc():
    if "nc" not in _CACHE:
        _CACHE["nc"] = _build()
    return _CACHE["nc"]


def _prep_inputs(x, adj, W_heads, a_heads, W_att, a_att, W1, W2, W3, eps):
    f32 = np.float32
    x = np.asarray(x, f32)
    adj = np.asarray(adj, f32)
    W_heads = np.asarray(W_heads, f32)
    a_heads = np.asarray(a_heads, f32)
    W_att = np.asarray(W_att, f32)
    a_att = np.asarray(a_att, f32)
    W1 = np.asarray(W1, f32)
    W2 = np.asarray(W2, f32)
    W3 = np.asarray(W3, f32)
    eps = np.asarray(eps, f32)

    xT = np.ascontiguousarray(x.T)
    rhsh = np.ascontiguousarray(
        np.stack([np.concatenate([W_heads[h], W_heads[h] @ a_heads[h, F1:, :]], 1)
                  for h in range(H)]))
    wa1 = np.ascontiguousarray(
        np.concatenate([W_heads[h] @ a_heads[h, :F1, :] for h in range(H)], 1))
    rhsa = np.ascontiguousarray(
        np.concatenate([W_att, W_att @ a_att[F1:], W_att @ a_att[:F1]], 1))
    w23 = np.ascontiguousarray(np.concatenate([W2, W3], 1))

    in_maps = []
    for k in range(NCORES):
        rows = slice(k * R, (k + 1) * R)
        in_maps.append({
            "adjT": np.ascontiguousarray(adj[rows].T),
            "xT": xT,
            "xTl": np.ascontiguousarray(x[rows].T),
            "rhsh": rhsh,
            "wa1": wa1,
            "rhsa": rhsa,
            "w1": W1,
            "w23": w23,
            "epsl": np.ascontiguousarray(eps[rows]),
        })
    return in_maps


def _run(in_maps, trace=False):
    from concourse.bass_utils import run_bass_kernel_spmd
    nc = _get_nc()
    res = run_bass_kernel_spmd(nc, in_maps, core_ids=list(range(NCORES)),
                               trace=trace)
    return res


def kernel(x, adj, W_heads, a_heads, W_att, a_att, W1, W2, W3, eps):
    in_maps = _prep_inputs(x, adj, W_heads, a_heads, W_att, a_att, W1, W2, W3, eps)
    res = _run(in_maps)
    adj_rec = np.concatenate([res.results[k]["out_rec"] for k in range(NCORES)], 0)
    mu = np.concatenate([res.results[k]["out_mu"] for k in range(NCORES)], 0)
    logvar = np.concatenate([res.results[k]["out_lv"] for k in range(NCORES)], 0)
    return adj_rec, mu, logvar
